# revision 1
# baseline (speedup 1.0000x reference)
"""Trainium2 Bass kernel for 2-layer LSTM (T=512, B=32, H=512), fp32 I/O.

Strategy: pure data-parallel over batch (8 cores x B_local=4, zero collectives).
Each core runs the full 2-layer scan for its batch slice.

Per-core design:
  - All on-chip layouts are "chunk-partitioned": SBUF/PSUM partition index
    p = 32*k + 4*r + b, where k = hidden-unit chunk (u div 128), r = replica
    (batch replicated 8x to fill the 128-wide PE stationary / all engine
    lanes), b = local batch. Free dim carries (gate, u_lo) for z-shaped
    tiles and u_lo for state-shaped tiles.
  - z matmuls: stationary = transposed hidden state (with replicas), moving =
    pre-transposed weight matrices; 4 PE column-groups stream the 4
    unit-chunks concurrently (tile_position col-tiling). A K=5 "inject"
    round adds x-projection + bias via a small identity-select stationary.
  - Gates: single ACT Tanh with scale=0.5 over [128, 512] using the
    tanh-half-trick (sigmoid(z) = 0.5*tanh(z/2)+0.5); the doubled-gate
    row scaling is folded into the weights on the host. Cell state is kept
    as S := 2c and hidden as H := 2h so the whole cell update is 4
    scalar_tensor_tensor ops; host folds the compensating 0.5 into all
    H-consuming weight columns and rescales the final output.
  - x-projections are big GEMMs: layer-1 fully precomputed to DRAM before
    the scan; layer-2 computed in 8-step blocks from transposed H1 tiles
    while the scan runs (layer-2 scan lags layer-1 by LAG=16 steps).
  - The next step's stationary comes from a PE transpose of H + a PSUM->SBUF
    copy; the replicated layout is self-consistent under this transpose.
"""

import sys

if "/opt/trn_rl_repo" not in sys.path:
    sys.path.insert(0, "/opt/trn_rl_repo")

import numpy as np
import ml_dtypes

import concourse.bacc as bacc
import concourse.tile as tile
from concourse import mybir
from concourse.bass_utils import run_bass_kernel_spmd

T_STEPS = 512
B_LOC = 4
N_CORES = 8
H = 512
NG = 2048  # 4*H gate width
BLK = 8  # x2proj block size (steps)
LAG = 16  # layer-2 scan lag (steps)
BF = mybir.dt.bfloat16
F32 = mybir.dt.float32

_ADD = mybir.AluOpType.add
_MUL = mybir.AluOpType.mult
_TANH = mybir.ActivationFunctionType.Tanh


def _eff_stream(w, col_scale):
    """w: [2048, d] (gate-major rows i,f,g,o). Returns stream matrix
    [d, 2048] with rows scaled (g-gate x2 for the tanh trick), columns
    reordered chunk-major (n = 512*k + 128*g + u_lo), scaled by col_scale."""
    w2 = w.astype(np.float64) * col_scale
    w2[2 * H : 3 * H] *= 2.0
    wr = w2.reshape(4, 4, 128, w.shape[1])  # [g, k, u_lo, d]
    wr = wr.transpose(1, 0, 2, 3).reshape(NG, w.shape[1])
    return np.ascontiguousarray(wr.T.astype(np.float32))


def _eff_bias(b_ih, b_hh):
    b = (b_ih.astype(np.float64) + b_hh.astype(np.float64)).copy()
    b[2 * H : 3 * H] *= 2.0
    br = b.reshape(4, 4, 128).transpose(1, 0, 2).reshape(NG)
    return br.astype(np.float32)


def _bf16(x):
    return x.astype(ml_dtypes.bfloat16)


def _z_rounds(nc, pz, is5, xp_sl, ht, w):
    """Emit the 5 col-tiled accumulation rounds for one z = xproj+b+W.T@H."""
    for j in range(4):
        osl = pz[32 * j : 32 * j + 32, :]
        nc.tensor.matmul(
            osl,
            is5[:, :],
            xp_sl[0:4, 512 * j : 512 * j + 512],
            start=True,
            stop=False,
            tile_position=(0, 32 * j),
        )
        for k in range(4):
            nc.tensor.matmul(
                osl,
                ht[:, 32 * k : 32 * k + 32],
                w[:, NG * k + 512 * j : NG * k + 512 * j + 512],
                start=False,
                stop=(k == 3),
                tile_position=(0, 32 * j),
            )


def _gates(nc, pools, pz, s_prev, h_dtype, nm):
    """ACT+DVE cell update. Returns (s_new, h_new) tiles."""
    sbuf, state = pools
    t_sb = sbuf.tile([128, 512], BF, tag=f"t{nm}", name=f"t{nm}")
    nc.scalar.activation(t_sb[:], pz[:], _TANH, bias=0.0, scale=0.5)
    m2 = sbuf.tile([128, 128], F32, tag=f"m2{nm}", name=f"m2{nm}")
    nc.vector.scalar_tensor_tensor(
        m2[:], t_sb[:, 0:128], 1.0, t_sb[:, 256:384], op0=_ADD, op1=_MUL
    )
    m1 = sbuf.tile([128, 128], F32, tag=f"m1{nm}", name=f"m1{nm}")
    nc.vector.scalar_tensor_tensor(
        m1[:], t_sb[:, 128:256], 1.0, s_prev[:], op0=_ADD, op1=_MUL
    )
    s_new = state.tile([128, 128], F32, tag=f"s{nm}", name=f"s{nm}")
    nc.vector.scalar_tensor_tensor(
        s_new[:], m1[:], 0.5, m2[:], op0=_MUL, op1=_ADD
    )
    tc_sb = sbuf.tile([128, 128], BF, tag=f"tc{nm}", name=f"tc{nm}")
    nc.scalar.activation(tc_sb[:], s_new[:], _TANH, bias=0.0, scale=0.5)
    h_new = sbuf.tile([128, 128], h_dtype, tag=f"h{nm}", name=f"h{nm}")
    nc.vector.scalar_tensor_tensor(
        h_new[:], t_sb[:, 384:512], 1.0, tc_sb[:], op0=_ADD, op1=_MUL
    )
    return s_new, h_new


def build_nc(t_steps=T_STEPS, repeat=1):
    nc = bacc.Bacc(
        "TRN2", target_bir_lowering=False, debug=False, num_devices=N_CORES
    )
    # kernel inputs (per-core)
    xt_d = nc.dram_tensor("xt", [H, t_steps * B_LOC], BF, kind="ExternalInput")
    w1i_d = nc.dram_tensor("w1i", [H, NG], BF, kind="ExternalInput")
    w1h_d = nc.dram_tensor("w1h", [H, NG], BF, kind="ExternalInput")
    w2i_d = nc.dram_tensor("w2i", [H, NG], BF, kind="ExternalInput")
    w2h_d = nc.dram_tensor("w2h", [H, NG], BF, kind="ExternalInput")
    b1_d = nc.dram_tensor("b1r", [1, NG], BF, kind="ExternalInput")
    b2_d = nc.dram_tensor("b2r", [1, NG], BF, kind="ExternalInput")
    is5_d = nc.dram_tensor("is4", [4, 32], BF, kind="ExternalInput")
    idb_d = nc.dram_tensor("idb", [128, 128], BF, kind="ExternalInput")
    idf_d = nc.dram_tensor("idf", [128, 128], F32, kind="ExternalInput")
    y_d = nc.dram_tensor("y", [t_steps, 2048], F32, kind="ExternalOutput")

    rb_sz = min(128, t_steps * B_LOC)  # phase-1 row-block size
    n_tb = t_steps * B_LOC // rb_sz

    with tile.TileContext(nc) as tc:
        with (
            tc.tile_pool(name="const", bufs=1) as const,
            tc.tile_pool(name="state", bufs=2) as state,
            tc.tile_pool(name="work", bufs=2) as work,
            tc.tile_pool(name="xp1p", bufs=3) as xp1p,
            tc.tile_pool(name="xp2p", bufs=3) as xp2p,
            tc.tile_pool(name="hblkp", bufs=2) as hblkp,
            tc.tile_pool(name="pzp", bufs=1, space="PSUM") as pzp,
            tc.tile_pool(name="ptp", bufs=1, space="PSUM") as ptp,
            tc.tile_pool(name="dram", bufs=1, space="DRAM") as dramp,
        ):
            # ---- constants / weights to SBUF
            is5 = const.tile([4, 32], BF, name="is4")
            nc.sync.dma_start(is5[:], is5_d.ap())
            b1sb = const.tile([1, NG], BF, name="b1sb")
            nc.sync.dma_start(b1sb[:], b1_d.ap())
            b2sb = const.tile([1, NG], BF, name="b2sb")
            nc.sync.dma_start(b2sb[:], b2_d.ap())
            ones1 = const.tile([1, 128], BF, name="ones1")
            nc.gpsimd.memset(ones1[:], 1.0)
            idb = const.tile([128, 128], BF)
            nc.sync.dma_start(idb[:], idb_d.ap())
            idf = const.tile([128, 128], F32)
            nc.sync.dma_start(idf[:], idf_d.ap())

            w1h = const.tile([128, 4 * NG], BF, name="w1h")
            w2i = const.tile([128, 4 * NG], BF, name="w2i")
            w2h = const.tile([128, 4 * NG], BF, name="w2h")
            for w_sb, w_dd in ((w1h, w1h_d), (w2i, w2i_d), (w2h, w2h_d)):
                for k in range(4):
                    nc.sync.dma_start(
                        w_sb[:, NG * k : NG * k + NG],
                        w_dd[128 * k : 128 * k + 128, :],
                    )

            # DRAM scratch
            x1d = dramp.tile([t_steps * B_LOC, NG], BF, name="x1d")
            x2d = dramp.tile([t_steps * B_LOC, NG], BF, name="x2d")

            for _rep in range(repeat):
              # ---- phase 1: x1proj GEMM -> DRAM
              with (
                  tc.tile_pool(name="ph1", bufs=2) as ph1,
                  tc.tile_pool(name="ph1ps", bufs=2, space="PSUM") as ph1ps,
              ):
                  w1i = ph1.tile([128, 4 * NG], BF, bufs=1, name="w1i")
                  xts = ph1.tile([128, 4 * t_steps * B_LOC], BF, bufs=1,
                                 name="xts")
                  for k in range(4):
                      nc.sync.dma_start(
                          w1i[:, NG * k : NG * k + NG],
                          w1i_d[128 * k : 128 * k + 128, :],
                      )
                      nc.sync.dma_start(
                          xts[:, t_steps * B_LOC * k : t_steps * B_LOC * (k + 1)],
                          xt_d[128 * k : 128 * k + 128, :],
                      )
                  for tb in range(n_tb):
                      cpx = ph1.tile([rb_sz, NG], BF, name="cpx")
                      for nj in range(4):
                          pxa = ph1ps.tile([rb_sz, 512], F32, name="pxa")
                          nc.tensor.matmul(
                              pxa[:],
                              ones1[0:1, 0:rb_sz],
                              b1sb[0:1, 512 * nj : 512 * nj + 512],
                              start=True,
                              stop=False,
                          )
                          for k in range(4):
                              nc.tensor.matmul(
                                  pxa[:],
                                  xts[:, t_steps * B_LOC * k + rb_sz * tb :
                                      t_steps * B_LOC * k + rb_sz * tb + rb_sz],
                                  w1i[:, NG * k + 512 * nj : NG * k + 512 * nj + 512],
                                  start=False,
                                  stop=(k == 3),
                              )
                          nc.vector.tensor_copy(
                              cpx[:, 512 * nj : 512 * nj + 512], pxa[:]
                          )
                      nc.sync.dma_start(
                          x1d[rb_sz * tb : rb_sz * tb + rb_sz, :], cpx[:]
                      )

              # ---- initial states
              s1 = state.tile([128, 128], F32, tag="s1", name="s1")
              nc.gpsimd.memset(s1[:], 0.0)
              s2 = state.tile([128, 128], F32, tag="s2", name="s2")
              nc.gpsimd.memset(s2[:], 0.0)
              ht1 = state.tile([128, 128], BF, tag="ht1", name="ht1")
              nc.gpsimd.memset(ht1[:], 0.0)
              ht2 = state.tile([128, 128], BF, tag="ht2", name="ht2")
              nc.gpsimd.memset(ht2[:], 0.0)

              xp1 = xp2 = hblk = None
              pools = (work, state)

              # ---- fused scan
              for tau in range(t_steps + LAG):
                  if tau < t_steps:
                      t1i = tau
                      bi, off = divmod(t1i, BLK)
                      if off == 0:
                          hblk = hblkp.tile([128, BLK * 16], BF, tag="hblk",
                                            name="hblk")
                      xp1 = xp1p.tile([B_LOC, NG], BF, tag="xp1", name="xp1")
                      nc.sync.dma_start(
                          xp1[:], x1d[B_LOC * t1i : B_LOC * t1i + B_LOC, :]
                      )
                      pz1 = pzp.tile([128, 512], F32, tag="pz1", name="pz1")
                      _z_rounds(nc, pz1, is5, xp1, ht1, w1h)
                      s1, h1 = _gates(nc, pools, pz1, s1, BF, "L1")
                      pt1 = ptp.tile([128, 128], BF, tag="pt1", name="pt1")
                      nc.tensor.transpose(pt1[:], h1[:], idb[:])
                      ht1 = state.tile([128, 128], BF, tag="ht1", name="ht1")
                      nc.vector.tensor_copy(ht1[:], pt1[:])
                      nc.vector.tensor_copy(
                          hblk[:].rearrange("p (k t b) -> p k t b", k=4, b=4)
                          [:, :, off, :],
                          pt1[:].rearrange("p (k rb) -> p k rb", rb=32)[:, :, 0:4],
                      )
                      if off == BLK - 1:
                          # x2proj GEMM for this block
                          hbr = hblk[:].rearrange("p (k tb) -> p k tb", k=4)
                          cx2 = work.tile([32, NG], BF, tag="cx2", name="cx2")
                          for hf in range(2):
                              pxb = pzp.tile([32, 1024], F32, tag="pxb",
                                             name="pxb")
                              for nj2 in range(2):
                                  nj = 2 * hf + nj2
                                  nc.tensor.matmul(
                                      pxb[:, 512 * nj2 : 512 * nj2 + 512],
                                      ones1[0:1, 0:32],
                                      b2sb[0:1, 512 * nj : 512 * nj + 512],
                                      start=True,
                                      stop=False,
                                  )
                                  for k in range(4):
                                      nc.tensor.matmul(
                                          pxb[:, 512 * nj2 : 512 * nj2 + 512],
                                          hbr[:, k, :],
                                          w2i[:, NG * k + 512 * nj :
                                              NG * k + 512 * nj + 512],
                                          start=False,
                                          stop=(k == 3),
                                      )
                              nc.vector.tensor_copy(
                                  cx2[:, 1024 * hf : 1024 * hf + 1024], pxb[:]
                              )
                          nc.sync.dma_start(
                              x2d[B_LOC * BLK * bi : B_LOC * BLK * (bi + 1), :],
                              cx2[:],
                          )
                  if tau >= LAG:
                      t2i = tau - LAG
                      xp2 = xp2p.tile([B_LOC, NG], BF, tag="xp2", name="xp2")
                      nc.sync.dma_start(
                          xp2[:], x2d[B_LOC * t2i : B_LOC * t2i + B_LOC, :]
                      )
                      pz2 = pzp.tile([128, 512], F32, tag="pz2", name="pz2")
                      _z_rounds(nc, pz2, is5, xp2, ht2, w2h)
                      s2, h2 = _gates(nc, pools, pz2, s2, F32, "L2")
                      pt2 = ptp.tile([128, 128], F32, tag="pt2", name="pt2")
                      nc.tensor.transpose(pt2[:], h2[:], idf[:])
                      ht2 = state.tile([128, 128], BF, tag="ht2", name="ht2")
                      nc.vector.tensor_copy(ht2[:], pt2[:])
                      yst = work.tile([128, 16], F32, tag="yst", name="yst")
                      nc.vector.tensor_copy(
                          yst[:].rearrange("u (k b) -> u k b", b=4),
                          pt2[:].rearrange("u (k rb) -> u k rb", rb=32)[:, :, 0:4],
                      )
                      nc.sync.dma_start(
                          y_d[t2i : t2i + 1, :]
                          .rearrange("o (u f) -> (o u) f", u=128),
                          yst[:],
                      )

    nc.compile()
    return nc


def host_inputs(seq_inputs, W_ih, W_hh, b_ih, b_hh, t_steps=T_STEPS):
    """Build the 8 per-core input maps."""
    w1i = _bf16(_eff_stream(W_ih[0], 1.0))
    w1h = _bf16(_eff_stream(W_hh[0], 0.5))
    w2i = _bf16(_eff_stream(W_ih[1], 0.5))
    w2h = _bf16(_eff_stream(W_hh[1], 0.5))
    b1 = _bf16(_eff_bias(b_ih[0], b_hh[0]))[None, :]
    b2 = _bf16(_eff_bias(b_ih[1], b_hh[1]))[None, :]
    is4 = np.zeros((4, 32), np.float32)
    for r in range(8):
        for b in range(B_LOC):
            is4[b, 4 * r + b] = 1.0
    is4 = _bf16(is4)
    idb = _bf16(np.eye(128, dtype=np.float32))
    idf = np.eye(128, dtype=np.float32)

    in_maps = []
    for c in range(N_CORES):
        xs = seq_inputs[:t_steps, B_LOC * c : B_LOC * (c + 1), :]  # [T,4,512]
        xt = np.ascontiguousarray(
            xs.transpose(2, 0, 1).reshape(H, t_steps * B_LOC)
        )
        in_maps.append(
            {
                "xt": _bf16(xt),
                "w1i": w1i,
                "w1h": w1h,
                "w2i": w2i,
                "w2h": w2h,
                "b1r": b1,
                "b2r": b2,
                "is4": is4,
                "idb": idb,
                "idf": idf,
            }
        )
    return in_maps


def gather_output(results, t_steps=T_STEPS):
    B = B_LOC * N_CORES
    y = np.empty((t_steps, B, H), np.float32)
    for c in range(N_CORES):
        yc = results[c]["y"].reshape(t_steps, 128, 4, 4)  # [t, u_lo, k, b]
        yc = yc.transpose(0, 3, 2, 1).reshape(t_steps, B_LOC, H)
        y[:, B_LOC * c : B_LOC * (c + 1), :] = yc
    return 0.5 * y  # H2 = 2*h2


_NC_CACHE = {}


def kernel(seq_inputs, W_ih, W_hh, b_ih, b_hh):
    seq_inputs = np.asarray(seq_inputs, np.float32)
    W_ih = np.asarray(W_ih, np.float32)
    W_hh = np.asarray(W_hh, np.float32)
    b_ih = np.asarray(b_ih, np.float32)
    b_hh = np.asarray(b_hh, np.float32)
    t_steps = seq_inputs.shape[0]
    if t_steps not in _NC_CACHE:
        _NC_CACHE[t_steps] = build_nc(t_steps)
    nc = _NC_CACHE[t_steps]
    in_maps = host_inputs(seq_inputs, W_ih, W_hh, b_ih, b_hh, t_steps)
    res = run_bass_kernel_spmd(nc, in_maps, core_ids=list(range(N_CORES)))
    return gather_output(res.results, t_steps)



# revision 3
# speedup vs baseline: 4862.1445x; 4862.1445x over previous
"""Trainium2 Bass kernel for 2-layer LSTM (T=512, B=32, H=512), fp32 I/O.

v2: weights-stationary transposed-z design, data-parallel over batch
(8 cores x B_loc=4, zero collectives).

Per-core design (all bf16 compute, fp32 PSUM accumulation):
  - z is computed TRANSPOSED: zT[u_lo, (g, k_u, b)] = [128, 64] per step.
    For each gate-block (g, k_u) (128 gate-units) and contraction chunk
    kc, one matmul with stationary = weight chunk [128, 128] and moving =
    hT chunk [128, 4(b)].  16 blocks x 4 chunks = 64 tiny matmuls.
  - xp and bias are injected by two identity-stationary matmuls from
    SBUF-resident projection tables (start=True clears the psum region).
  - Gates: one ACT tanh [128, 64] (tanh-half trick, g-rows doubled in the
    host-folded weights), 3 STT ops [128, 16] for the cell update
    (S := 2c convention), one ACT tanh [128, 16], one STT for h.
    h lands directly in the hT layout [128, (k, b)] -- no transpose, no
    PSUM copy.  Consumers of H=2h get 0.5-folded weights; y scaled on
    host.
  - x1 projection: phase-1 GEMM (weights-stationary) into an SBUF-resident
    table x1T [128, (g, k_u, t, b)]; zero per-step DMA.
  - x2 projection: computed per 8-step block from h1 block tiles into an
    SBUF ring (no DRAM round-trip); layer-2 scan lags by LAG steps.
  - y: h2 blocks [128, (k, t8, b)] bf16 DMA'd out per 8 steps.
"""

import sys

if "/opt/trn_rl_repo" not in sys.path:
    sys.path.insert(0, "/opt/trn_rl_repo")

import numpy as np
import ml_dtypes

import concourse.bacc as bacc
import concourse.tile as tile
from concourse import mybir
from concourse.bass_utils import run_bass_kernel_spmd

T_STEPS = 512
B_LOC = 4
N_CORES = 8
H = 512
NG = 2048
BLK = 8  # x2proj block size (steps)
LAG = 16  # layer-2 scan lag (steps)
BF = mybir.dt.bfloat16
F32 = mybir.dt.float32

_ADD = mybir.AluOpType.add
_MUL = mybir.AluOpType.mult
_TANH = mybir.ActivationFunctionType.Tanh


def _bf16(x):
    return x.astype(ml_dtypes.bfloat16)


def _stat_w(w, col_scale):
    """w: [2048, 512] logical (rows 512g+u_out, cols u_in).  Returns the
    stationary table [128, (kc, g, k_u, u_out_lo)] = [128, 8192], with
    g-gate rows doubled (tanh trick) and everything scaled col_scale."""
    w2 = w.astype(np.float64) * col_scale
    w2[2 * H : 3 * H] *= 2.0
    # [g, k_u, u_out_lo, kc, u_in_lo]; gate order permuted to (i, f, o, g)
    wr = w2.reshape(4, 4, 128, 4, 128)[[0, 1, 3, 2]]
    wr = wr.transpose(4, 3, 0, 1, 2).reshape(128, 8192)
    return np.ascontiguousarray(wr.astype(np.float32))


def _bias_t(b_ih, b_hh):
    """Returns [128, (g, k_u, b)] = [128, 64] bias table (g-rows doubled),
    replicated over the 4 batch columns."""
    b = (b_ih.astype(np.float64) + b_hh.astype(np.float64)).copy()
    b[2 * H : 3 * H] *= 2.0
    bt = b.reshape(4, 4, 128)[[0, 1, 3, 2]].transpose(2, 0, 1)  # [u_lo, g, k_u]
    bt = np.repeat(bt.reshape(128, 16, 1), B_LOC, axis=2).reshape(128, 64)
    return bt.astype(np.float32)


def _zstep(nc, pz, idb, xp_sl, bT, wst, hmov):
    """Emit inject + 64 accumulation matmuls for one zT = xp (+ b) + W.T@h.
    bT None means the bias is already folded into the xp table."""
    nc.tensor.matmul(pz[:], idb[:], xp_sl, start=True, stop=False)
    if bT is not None:
        nc.tensor.matmul(pz[:], idb[:], bT[:], start=False, stop=False)
    for blk in range(16):
        osl = pz[:, 4 * blk : 4 * blk + 4]
        for kc in range(4):
            nc.tensor.matmul(
                osl,
                wst[:, 2048 * kc + 128 * blk : 2048 * kc + 128 * blk + 128],
                hmov(kc),
                start=False,
                stop=(kc == 3),
            )


def _gates_a(nc, sbuf, pz, t_tile, nm, eng):
    """First gate phase: big tanh + fused cell products + new cell state.
    t_tile [128, 96] = [i f o g | s_prev | tc] (s_prev from the previous
    step).  Gate rows in the weight tables are permuted (i,f,o,g) so one
    fused STT computes both cell products:
      m12 = ((i|f)+1) * (g|s_prev) = (2i*g | 2f*S_prev)
      S_new = 0.5*m12_hi + m12_lo  -> next t_tile[64:80] (bf16)
    Returns the next-step t_tile."""
    nc.scalar.activation(t_tile[:, 0:64], pz[:], _TANH, bias=0.0, scale=0.5)
    t_next = sbuf.tile([128, 96], BF, tag=f"t{nm}2", name=f"t{nm}2")
    m12 = sbuf.tile([128, 32], BF, tag=f"m12{nm}", name=f"m12{nm}")
    eng.scalar_tensor_tensor(
        m12[:], t_tile[:, 0:32], 1.0, t_tile[:, 48:80], op0=_ADD, op1=_MUL
    )
    eng.scalar_tensor_tensor(
        t_next[:, 64:80], m12[:, 16:32], 0.5, m12[:, 0:16],
        op0=_MUL, op1=_ADD,
    )
    return t_next


def _gates_b(nc, t_tile, t_next, hout_ap, eng):
    """Second gate phase: cell tanh + output gate -> h (doubled, bf16)."""
    nc.scalar.activation(
        t_next[:, 80:96], t_next[:, 64:80], _TANH, bias=0.0, scale=0.5
    )
    eng.scalar_tensor_tensor(
        hout_ap, t_tile[:, 32:48], 1.0, t_next[:, 80:96], op0=_ADD, op1=_MUL
    )


def build_nc(t_steps=T_STEPS, repeat=1):
    nc = bacc.Bacc(
        "TRN2", target_bir_lowering=False, debug=False, num_devices=N_CORES
    )
    n_blk = t_steps // BLK
    # kernel inputs (per-core)
    xt_d = nc.dram_tensor("xt", [H, t_steps * B_LOC], BF, kind="ExternalInput")
    w1i_d = nc.dram_tensor("w1i", [128, 8192], BF, kind="ExternalInput")
    w1h_d = nc.dram_tensor("w1h", [128, 8192], BF, kind="ExternalInput")
    w2i_d = nc.dram_tensor("w2i", [128, 8192], BF, kind="ExternalInput")
    w2h_d = nc.dram_tensor("w2h", [128, 8192], BF, kind="ExternalInput")
    b1_d = nc.dram_tensor("b1t", [128, 64], BF, kind="ExternalInput")
    b2_d = nc.dram_tensor("b2t", [128, 64], BF, kind="ExternalInput")
    idb_d = nc.dram_tensor("idb", [128, 128], BF, kind="ExternalInput")
    y_d = nc.dram_tensor("y", [n_blk, 128, 128], BF, kind="ExternalOutput")

    with tile.TileContext(nc) as tc:
        with (
            tc.tile_pool(name="const", bufs=1) as const,
            tc.tile_pool(name="state", bufs=2) as state,
            tc.tile_pool(name="work", bufs=3) as work,
            tc.tile_pool(name="h1p", bufs=3) as h1p,
            tc.tile_pool(name="h2p", bufs=3) as h2p,
            tc.tile_pool(name="x2p", bufs=4) as x2p,
            tc.tile_pool(name="pz1p", bufs=2, space="PSUM") as pz1p,
            tc.tile_pool(name="pz2p", bufs=2, space="PSUM") as pz2p,
            tc.tile_pool(name="px2p", bufs=2, space="PSUM") as px2p,
        ):
            # ---- constants / weights to SBUF
            idb = const.tile([128, 128], BF, name="idb")
            nc.sync.dma_start(idb[:], idb_d.ap())
            b1T = const.tile([128, 64], BF, name="b1T")
            nc.sync.dma_start(b1T[:], b1_d.ap())
            b2T = const.tile([128, 64], BF, name="b2T")
            nc.sync.dma_start(b2T[:], b2_d.ap())
            w1h = const.tile([128, 8192], BF, name="w1h")
            w2h = const.tile([128, 8192], BF, name="w2h")
            w2i = const.tile([128, 8192], BF, name="w2i")

            # x1T: SBUF-resident layer-1 input projections
            # [128, (g, k_u, t, b)]; strides g: 16*tB, k_u: 4*tB, t: 4, b: 1
            tB = t_steps * B_LOC
            x1T = const.tile([128, 16 * tB], BF, name="x1T")

            for _rep in range(repeat):
              # ---- phase 1: x1 projection GEMM into x1T
              with (
                  tc.tile_pool(name="ph1", bufs=1) as ph1,
                  tc.tile_pool(name="ph1ps", bufs=2, space="PSUM") as ph1ps,
              ):
                  w1i = ph1.tile([128, 8192], BF, name="w1i")
                  nc.sync.dma_start(w1i[:], w1i_d.ap())
                  xts = ph1.tile([128, 8192], BF, name="xts")
                  for kc in range(4):
                      nc.sync.dma_start(
                          xts[:, 2048 * kc : 2048 * kc + 2048],
                          xt_d[128 * kc : 128 * kc + 128, :],
                      )
                  # scan weights load during phase-1 (after its inputs)
                  if _rep == 0:
                      nc.sync.dma_start(w1h[:], w1h_d.ap())
                      nc.sync.dma_start(w2h[:], w2h_d.ap())
                      nc.sync.dma_start(w2i[:], w2i_d.ap())
                  n_tblk = tB // 512
                  for tb in range(n_tblk):
                      for blk in range(16):
                          pxa = ph1ps.tile([128, 512], F32, name="pxa")
                          for kc in range(4):
                              nc.tensor.matmul(
                                  pxa[:],
                                  w1i[:, 2048 * kc + 128 * blk :
                                      2048 * kc + 128 * blk + 128],
                                  xts[:, 2048 * kc + 512 * tb :
                                      2048 * kc + 512 * tb + 512],
                                  start=(kc == 0),
                                  stop=(kc == 3),
                              )
                          # copy + fold b1 (per-partition bias for this block)
                          nc.scalar.activation(
                              x1T[:, tB * blk + 512 * tb :
                                  tB * blk + 512 * tb + 512],
                              pxa[:],
                              mybir.ActivationFunctionType.Identity,
                              bias=b1T[:, 4 * blk : 4 * blk + 1],
                              scale=1.0,
                          )

              # ---- initial states
              t1 = work.tile([128, 96], BF, tag="tL12", name="tL1i")
              nc.gpsimd.memset(t1[:, 64:80], 0.0)
              t2 = work.tile([128, 96], BF, tag="tL22", name="tL2i")
              nc.gpsimd.memset(t2[:, 64:80], 0.0)
              h1z = state.tile([128, 16], BF, tag="h1z", name="h1z")
              nc.gpsimd.memset(h1z[:], 0.0)
              h2z = state.tile([128, 16], BF, tag="h2z", name="h2z")
              nc.gpsimd.memset(h2z[:], 0.0)

              hblk1 = prev_hblk1 = None
              yblk = prev_yblk = None
              cx2 = {}

              def h1mov(tau):
                  """Moving AP maker for layer-1 h at step tau-1."""
                  if tau == 0:
                      return lambda kc: h1z[:, 0:4]
                  off = (tau - 1) % BLK
                  # tau-1 in current block unless tau % BLK == 0
                  tile_ = prev_hblk1 if tau % BLK == 0 else hblk1
                  return lambda kc: tile_[
                      :, 32 * kc + 4 * off : 32 * kc + 4 * off + 4
                  ]

              def h2mov(t2):
                  if t2 == 0:
                      return lambda kc: h2z[:, 0:4]
                  off = (t2 - 1) % BLK
                  tile_ = prev_yblk if t2 % BLK == 0 else yblk
                  return lambda kc: tile_[
                      :, 32 * kc + 4 * off : 32 * kc + 4 * off + 4
                  ]

              # ---- fused scan
              for tau in range(t_steps + LAG):
                  if tau < t_steps:
                      bi, off = divmod(tau, BLK)
                      if off == 0:
                          prev_hblk1 = hblk1
                          hblk1 = h1p.tile([128, 128], BF, tag="hb1",
                                           name="hb1")
                      hm = h1mov(tau)
                      pz1 = pz1p.tile([128, 64], F32, tag="pz1", name="pz1")
                      _zstep(
                          nc, pz1, idb,
                          x1T[:, :].rearrange(
                              "p (gk tb) -> p gk tb", gk=16
                          )[:, :, 4 * tau : 4 * tau + 4],
                          None, w1h, hm,
                      )
                      t1n = _gates_a(nc, work, pz1, t1, "L1", nc.vector)
                  if tau >= LAG:
                      st2 = tau - LAG
                      b2i, off2 = divmod(st2, BLK)
                      if off2 == 0:
                          prev_yblk = yblk
                          yblk = h2p.tile([128, 128], BF, tag="yb",
                                          name="yb")
                      hm2 = h2mov(st2)
                      cxt = cx2[b2i]
                      pz2 = pz2p.tile([128, 64], F32, tag="pz2", name="pz2")
                      _zstep(
                          nc, pz2, idb,
                          cxt[:, :].rearrange(
                              "p (gk tb) -> p gk tb", gk=16
                          )[:, :, 4 * off2 : 4 * off2 + 4],
                          b2T, w2h, hm2,
                      )
                      t2n = _gates_a(nc, work, pz2, t2, "L2", nc.vector)
                  if tau < t_steps:
                      _gates_b(
                          nc, t1, t1n,
                          hblk1[:, :].rearrange(
                              "p (k t b) -> p k t b", k=4, t=BLK
                          )[:, :, off, :],
                          nc.vector,
                      )
                      t1 = t1n
                  if tau >= LAG:
                      _gates_b(
                          nc, t2, t2n,
                          yblk[:, :].rearrange(
                              "p (k t b) -> p k t b", k=4, t=BLK
                          )[:, :, off2, :],
                          nc.vector,
                      )
                      t2 = t2n
                      if off2 == BLK - 1:
                          nc.sync.dma_start(y_d[b2i], yblk[:])
                  if tau < t_steps and tau % BLK == BLK - 1:
                      bi = tau // BLK
                      # x2 projection for the just-finished layer-1 block
                      px2 = px2p.tile([128, 512], F32, tag="px2", name="px2")
                      for blk2 in range(16):
                          for kc in range(4):
                              nc.tensor.matmul(
                                  px2[:, 32 * blk2 : 32 * blk2 + 32],
                                  w2i[:, 2048 * kc + 128 * blk2 :
                                      2048 * kc + 128 * blk2 + 128],
                                  hblk1[:, 32 * kc : 32 * kc + 32],
                                  start=(kc == 0),
                                  stop=(kc == 3),
                              )
                      cx = x2p.tile([128, 512], BF, tag="cx2", name="cx2")
                      nc.vector.tensor_copy(cx[:], px2[:])
                      cx2[bi] = cx

    nc.compile()
    return nc


def host_inputs(seq_inputs, W_ih, W_hh, b_ih, b_hh, t_steps=T_STEPS):
    """Build the 8 per-core input maps."""
    w1i = _bf16(_stat_w(W_ih[0], 1.0))
    w1h = _bf16(_stat_w(W_hh[0], 0.5))
    w2i = _bf16(_stat_w(W_ih[1], 0.5))
    w2h = _bf16(_stat_w(W_hh[1], 0.5))
    b1 = _bf16(_bias_t(b_ih[0], b_hh[0]))
    b2 = _bf16(_bias_t(b_ih[1], b_hh[1]))
    idb = _bf16(np.eye(128, dtype=np.float32))

    in_maps = []
    for c in range(N_CORES):
        xs = seq_inputs[:t_steps, B_LOC * c : B_LOC * (c + 1), :]  # [T,4,512]
        xt = np.ascontiguousarray(
            xs.transpose(2, 0, 1).reshape(H, t_steps * B_LOC)
        )
        in_maps.append(
            {
                "xt": _bf16(xt),
                "w1i": w1i,
                "w1h": w1h,
                "w2i": w2i,
                "w2h": w2h,
                "b1t": b1,
                "b2t": b2,
                "idb": idb,
            }
        )
    return in_maps


def gather_output(results, t_steps=T_STEPS):
    B = B_LOC * N_CORES
    n_blk = t_steps // BLK
    y = np.empty((t_steps, B, H), np.float32)
    for c in range(N_CORES):
        yc = results[c]["y"].astype(np.float32)  # [n_blk, 128, 128]
        yc = yc.reshape(n_blk, 128, 4, BLK, B_LOC)  # [blk, u_lo, k, t8, b]
        yc = yc.transpose(0, 3, 4, 2, 1).reshape(t_steps, B_LOC, H)
        y[:, B_LOC * c : B_LOC * (c + 1), :] = yc
    return 0.5 * y  # H2 = 2*h2


_NC_CACHE = {}


def kernel(seq_inputs, W_ih, W_hh, b_ih, b_hh):
    seq_inputs = np.asarray(seq_inputs, np.float32)
    W_ih = np.asarray(W_ih, np.float32)
    W_hh = np.asarray(W_hh, np.float32)
    b_ih = np.asarray(b_ih, np.float32)
    b_hh = np.asarray(b_hh, np.float32)
    t_steps = seq_inputs.shape[0]
    if t_steps not in _NC_CACHE:
        _NC_CACHE[t_steps] = build_nc(t_steps)
    nc = _NC_CACHE[t_steps]
    in_maps = host_inputs(seq_inputs, W_ih, W_hh, b_ih, b_hh, t_steps)
    res = run_bass_kernel_spmd(nc, in_maps, core_ids=list(range(N_CORES)))
    return gather_output(res.results, t_steps)
